# revision 1
# baseline (speedup 1.0000x reference)
"""Trainium2 Bass kernel for nn_Net_17532056502451.

5 "think" iterations: shift-window cosine selector (159 shifts) + softmax
attention + scatter-back + conv-style encoder/decoder with energy argmax
(81 shifts), masked-MSE losses averaged.  Data-parallel: 1024 tokens over
8 cores, 128 tokens/core (one per SBUF partition), token-major fp32.

Mappings per core:
- dot correlation: 80 fused scalar_tensor_tensor MACs (DVE).
- sliding norms: Square + prefix-scan + strided diff.
- argmaxes: nc.vector.max / max_index (first-occurrence ties = jnp.argmax).
- per-token dynamic windows: GPSIMD ap_gather (idx[p,j] = off_p + j wrap
  trick, 16 candidate lanes) + 16-way predicated-copy diagonal extract.
- energy: quadratic Gram form. z[t,(d,i)] = ye[t,i]*ye[t,i+d] in ONE DVE op
  (overlapping APs), contracted with host-precomputed A via PE
  transpose -> PSUM->SBUF DMA -> accumulating matmuls.
- encoder/decoder: shared-weight matmuls on yhat (y_att embedded at d*),
  biases folded into PSUM->SBUF activation copies.
"""
import numpy as np

IDIM = 80
ODIM = 80
HDIM = 512
THINK_ITER = 5
TEMPER = 0.7
B, T = 4, 256
NTOK = B * T
P = 128
NCORES = 8
S1 = 159
S2 = 81
NFEAT = 80 * 80
NCHUNK = NFEAT // 128   # 50

_cache = {}


def _build_consts(W_enc, b_enc, W_src, b_src):
    W_enc = np.asarray(W_enc, np.float32)
    b_enc = np.asarray(b_enc, np.float32)
    W_src = np.asarray(W_src, np.float32)
    b_src = np.asarray(b_src, np.float32)
    C = (W_enc.T @ W_enc).astype(np.float32)
    q = (W_enc.T @ b_enc).astype(np.float32)
    bb = np.float32(b_enc @ b_enc)
    # E[t,s] = sum_{d,i} Az[s, 80d+i] * ye_i ye_{i+d} + sum_i 2 q[dd+i] ye_i + bb,
    # dd = 80 - s
    Az = np.zeros((S2, NFEAT), np.float32)
    Al = np.zeros((S2, 81), np.float32)
    for s in range(S2):
        dd = 80 - s
        blk = C[dd:dd + 80, dd:dd + 80]
        for d in range(80):
            diag = np.diagonal(blk, offset=d).copy()
            Az[s, d * 80: d * 80 + (80 - d)] = (2.0 if d > 0 else 1.0) * diag
        Al[s, :80] = 2.0 * q[dd:dd + 80]
        Al[s, 80] = bb
    Az_cat = np.ascontiguousarray(Az.T)               # (6400, 81): pi-major
    Atail = np.ascontiguousarray(Al.T)                # (81, 81)
    W_encT = np.ascontiguousarray(W_enc.T)            # (160, 512)
    W_srcT = np.ascontiguousarray(W_src.T)            # (512, 160)
    M16 = np.zeros((P, 16), np.uint8)
    for p in range(P):
        M16[p, p % 16] = 1
    iota80 = np.broadcast_to(np.arange(80, dtype=np.float32), (P, 80)).copy()
    iota160 = np.broadcast_to(np.arange(160, dtype=np.float32), (P, 160)).copy()
    ident = np.eye(128, dtype=np.float32)
    benc4 = b_enc.reshape(4, 128).T.copy()            # (128, 4)
    bsrc2 = np.zeros((128, 2), np.float32)
    bsrc2[:, 0] = b_src[0:128]
    bsrc2[0:32, 1] = b_src[128:160]
    return dict(Az=Az_cat, Atail=Atail, WencT=W_encT, WsrcT=W_srcT,
                benc=benc4, bsrc=bsrc2, M16=M16, iota80=iota80,
                iota160=iota160, ident=ident,
                ones1=np.ones((1, 128), np.float32))


def _build_nc():
    import concourse.bass as bass
    import concourse.bacc as bacc
    import concourse.mybir as mybir
    from concourse.tile import TileContext

    F32 = mybir.dt.float32
    I16 = mybir.dt.int16
    U32 = mybir.dt.uint32
    Op = mybir.AluOpType
    AF = mybir.ActivationFunctionType

    nc = bacc.Bacc()
    d_x = nc.declare_dram_parameter("x", [P, 80], F32, isOutput=False)
    d_y = nc.declare_dram_parameter("y", [P, 80], F32, isOutput=False)
    d_A = nc.declare_dram_parameter("Az", [NFEAT, 81], F32, isOutput=False)
    d_At = nc.declare_dram_parameter("Atail", [81, 81], F32, isOutput=False)
    d_We = nc.declare_dram_parameter("WencT", [160, 512], F32, isOutput=False)
    d_Ws = nc.declare_dram_parameter("WsrcT", [512, 160], F32, isOutput=False)
    d_be = nc.declare_dram_parameter("benc", [128, 4], F32, isOutput=False)
    d_bs = nc.declare_dram_parameter("bsrc", [128, 2], F32, isOutput=False)
    d_M = nc.declare_dram_parameter("M16", [P, 16], mybir.dt.uint8, isOutput=False)
    d_i80 = nc.declare_dram_parameter("iota80", [P, 80], F32, isOutput=False)
    d_i160 = nc.declare_dram_parameter("iota160", [P, 160], F32, isOutput=False)
    d_id = nc.declare_dram_parameter("ident", [128, 128], F32, isOutput=False)
    d_on = nc.declare_dram_parameter("ones1", [1, 128], F32, isOutput=False)
    d_out = nc.declare_dram_parameter("losspart", [P, 8], F32, isOutput=True)

    with TileContext(nc) as tc:
        with (
            tc.tile_pool(name="const", bufs=1) as cpool,
            tc.tile_pool(name="work", bufs=1) as pool,
            tc.tile_pool(name="zrot", bufs=3) as zpool,
            tc.tile_pool(name="ps_rot", bufs=3, space="PSUM") as pp,
            tc.tile_pool(name="ps_acc", bufs=1, space="PSUM") as ppe,
        ):
            # ---- constants ----
            A_t = cpool.tile([P, NCHUNK * 81], F32, tag="A")
            for k in range(NCHUNK):
                nc.sync.dma_start(A_t[:, k * 81:(k + 1) * 81],
                                  d_A[k * 128:(k + 1) * 128, :])
            At_t = cpool.tile([81, 81], F32, tag="At")
            nc.sync.dma_start(At_t[:], d_At[:])
            We_t = cpool.tile([P, 2 * 512], F32, tag="We")
            nc.sync.dma_start(We_t[:, 0:512], d_We[0:128, :])
            nc.sync.dma_start(We_t[0:32, 512:1024], d_We[128:160, :])
            Ws_t = cpool.tile([P, 4 * 160], F32, tag="Ws")
            for k in range(4):
                nc.sync.dma_start(Ws_t[:, k * 160:(k + 1) * 160],
                                  d_Ws[k * 128:(k + 1) * 128, :])
            be_t = cpool.tile([128, 4], F32, tag="be")
            nc.sync.dma_start(be_t[:], d_be[:])
            bs_t = cpool.tile([128, 2], F32, tag="bs")
            nc.sync.dma_start(bs_t[:], d_bs[:])
            M_t = cpool.tile([P, 16], mybir.dt.uint8, tag="M")
            nc.sync.dma_start(M_t[:], d_M[:])
            i80_t = cpool.tile([P, 80], F32, tag="i80")
            nc.sync.dma_start(i80_t[:], d_i80[:])
            i160_t = cpool.tile([P, 160], F32, tag="i160")
            nc.sync.dma_start(i160_t[:], d_i160[:])
            id_t = cpool.tile([128, 128], F32, tag="id")
            nc.sync.dma_start(id_t[:], d_id[:])

            # ---- state ----
            xpad = pool.tile([P, 238], F32, tag="xpad")
            yres = pool.tile([P, 80], F32, tag="yres")
            keep = pool.tile([P, 80], F32, tag="keep")
            yap = pool.tile([P, 240], F32, tag="yap")
            lossp = pool.tile([P, 8], F32, tag="lossp")
            nc.vector.memset(xpad[:], 0.0)
            nc.vector.memset(yap[:], 0.0)
            nc.vector.memset(lossp[:], 0.0)
            nc.sync.dma_start(xpad[:, 79:159], d_x[:])
            nc.sync.dma_start(yres[:], d_y[:])
            nc.vector.tensor_scalar(keep[:], yres[:], 0.0, None, Op.not_equal)

            sqx = pool.tile([P, 239], F32, tag="sqx")
            nc.vector.memset(sqx[:, 0:1], 0.0)
            cs = pool.tile([P, 239], F32, tag="cs")
            nsq = pool.tile([P, S1], F32, tag="nsq")
            dot = pool.tile([P, S1], F32, tag="dot")
            adot = pool.tile([P, S1], F32, tag="adot")
            gsel = pool.tile([P, S1], F32, tag="gsel")
            rnsq = pool.tile([P, S1], F32, tag="rnsq")
            mx8 = pool.tile([P, 8], F32, tag="mx8")
            mi8 = pool.tile([P, 8], U32, tag="mi8")
            thf = pool.tile([P, 1], F32, tag="thf")
            idxf = pool.tile([P, 160], F32, tag="idxf")
            idxi = pool.tile([P, 160], I16, tag="idxi")
            g1280 = pool.tile([P, 1280], F32, tag="g1280")
            g2560 = pool.tile([P, 2560], F32, tag="g2560")
            yal = pool.tile([P, 80], F32, tag="yal")
            zt = pool.tile([P, 80], F32, tag="zt")
            et = pool.tile([P, 80], F32, tag="et")
            ssum = pool.tile([P, 1], F32, tag="ssum")
            rsum = pool.tile([P, 1], F32, tag="rsum")
            nzm = pool.tile([P, 1], F32, tag="nzm")
            zero1 = pool.tile([P, 1], F32, tag="zero1")
            nc.vector.memset(zero1[:], 0.0)
            xele = pool.tile([P, 80], F32, tag="xele")
            zfeat = pool.tile([P, NFEAT], F32, tag="zfeat")
            e81 = pool.tile([81, 128], F32, tag="e81")
            etail = pool.tile([81, 128], F32, tag="etail")
            nc.sync.dma_start(etail[80:81, :], d_on[:])
            Etok = pool.tile([P, S2], F32, tag="Etok")
            sf = pool.tile([P, 1], F32, tag="sf")
            df = pool.tile([P, 1], F32, tag="df")
            yhat = pool.tile([P, 160], F32, tag="yhat")
            yhT0 = pool.tile([128, 128], F32, tag="yhT0")
            yhT1 = pool.tile([32, 128], F32, tag="yhT1")
            hsT = pool.tile([128, 4 * 128], F32, tag="hsT")
            xeT0 = pool.tile([128, 128], F32, tag="xeT0")
            xeT1 = pool.tile([32, 128], F32, tag="xeT1")
            xext = pool.tile([P, 160], F32, tag="xext")
            yele = pool.tile([P, 80], F32, tag="yele")
            dtmp = pool.tile([P, 80], F32, tag="dtmp")

            ye_view = yap[:, 80:240]

            def gather_extract(src_ap, src_elems, width, out_tile, gbuf):
                """out[p, j] = src[p, idxf[p, j]], j in [0,width)."""
                nc.vector.tensor_copy(idxi[:, 0:width], idxf[:, 0:width])
                nc.gpsimd.ap_gather(gbuf[:, 0:16 * width], src_ap,
                                    idxi[:, 0:width], channels=128,
                                    num_elems=src_elems, d=1,
                                    num_idxs=16 * width)
                gv = gbuf[:, 0:16 * width].rearrange("p (j k) -> p j k", k=16)
                for k in range(16):
                    nc.vector.copy_predicated(
                        out_tile[:, 0:width],
                        M_t[:, k:k + 1].to_broadcast((P, width)),
                        gv[:, :, k])

            for it in range(THINK_ITER):
                # --- sliding norms ---
                nc.scalar.activation(sqx[:, 1:239], xpad[:], AF.Square)
                nc.vector.tensor_tensor_scan(cs[:], sqx[:],
                                             zero1[:].to_broadcast((P, 239)),
                                             0.0, Op.add, Op.bypass)
                nc.vector.tensor_tensor(nsq[:], cs[:, 80:239], cs[:, 0:159],
                                        Op.subtract)
                # --- dot: 80 MACs ---
                nc.vector.tensor_scalar_mul(dot[:], xpad[:, 0:S1], yres[:, 0:1])
                for c in range(1, 80):
                    nc.vector.scalar_tensor_tensor(dot[:], xpad[:, c:c + S1],
                                                   yres[:, c:c + 1], dot[:],
                                                   Op.mult, Op.add)
                # --- theta = argmax dot*|dot|/nsq ---
                nc.scalar.activation(adot[:], dot[:], AF.Abs)
                nc.vector.tensor_scalar_max(rnsq[:], nsq[:], 1e-30)
                nc.vector.reciprocal(rnsq[:], rnsq[:])
                nc.vector.tensor_tensor(gsel[:], dot[:], adot[:], Op.mult)
                nc.vector.tensor_tensor(gsel[:], gsel[:], rnsq[:], Op.mult)
                nc.vector.max(mx8[:], gsel[:])
                nc.vector.max_index(mi8[:], mx8[:], gsel[:])
                nc.vector.tensor_copy(thf[:], mi8[:, 0:1])
                # --- y_align gather ---
                nc.vector.scalar_tensor_tensor(idxf[:, 0:80], i80_t[:],
                                               thf[:, 0:1], i80_t[:],
                                               Op.add, Op.bypass)
                gather_extract(xpad[:], 238, 80, yal, g1280)
                # --- softmax attention -> y_att in yap[:, 80:160] ---
                nc.vector.tensor_tensor(zt[:], yal[:], yres[:], Op.mult)
                nc.vector.max(mx8[:], zt[:])
                nc.vector.tensor_scalar_mul(nzm[:], mx8[:, 0:1], -1.0 / TEMPER)
                nc.scalar.activation(et[:], zt[:], AF.Exp, bias=nzm[:, 0:1],
                                     scale=1.0 / TEMPER)
                nc.vector.tensor_reduce(ssum[:], et[:], mybir.AxisListType.X, Op.add)
                nc.vector.reciprocal(rsum[:], ssum[:])
                nc.vector.tensor_tensor(et[:], et[:], yal[:], Op.mult)
                nc.vector.tensor_scalar_mul(yap[:, 80:160], et[:], rsum[:, 0:1])
                # --- z features: z[p, 80d+i] = ye[i] * ye[i+d] ---
                in0 = ye_view[:, 0:80].unsqueeze(1).to_broadcast((P, 80, 80))
                in1 = bass.AP(ye_view.tensor, ye_view.offset,
                              [list(ye_view.ap[0]), [1, 80], [1, 80]])
                zv = zfeat[:].rearrange("p (d i) -> p d i", i=80)
                nc.vector.tensor_tensor(zv, in0, in1, Op.mult)
                # --- x_ele gather: idx = iota80 + (159 - theta) ---
                nc.vector.tensor_scalar_mul(thf[:], thf[:], -1.0)
                nc.vector.tensor_scalar_add(thf[:], thf[:], 159.0)
                nc.vector.scalar_tensor_tensor(idxf[:, 0:80], i80_t[:],
                                               thf[:, 0:1], i80_t[:],
                                               Op.add, Op.bypass)
                gather_extract(yap[:], 240, 80, xele, g1280)
                nc.vector.tensor_tensor(xpad[:, 79:159], xpad[:, 79:159],
                                        xele[:], Op.subtract)
                # --- E accumulation: pipelined T -> DMA -> MM ---
                Eps = ppe.tile([81, 128], F32, tag="Eps")
                zsb = [None] * NCHUNK
                for k in range(NCHUNK + 2):
                    if k < NCHUNK:
                        zTp = pp.tile([128, 128], F32, tag="zTp")
                        nc.tensor.transpose(zTp[:],
                                            zfeat[:, k * 128:(k + 1) * 128],
                                            id_t[:])
                        zsb_k = zpool.tile([128, 128], F32, tag="zT")
                        zsb[k] = zsb_k
                        nc.scalar.copy(zsb[k][:], zTp[:])
                    j = k - 2
                    if 0 <= j < NCHUNK:
                        nc.tensor.matmul(Eps[:], A_t[:, j * 81:(j + 1) * 81],
                                         zsb[j][:], start=(j == 0), stop=False)
                # tail: feats [ya(80); 1]
                yaTp = pp.tile([128, 128], F32, tag="zTp")
                nc.tensor.transpose(yaTp[0:80, :], yap[:, 80:160], id_t[:])
                nc.scalar.copy(etail[0:80, :], yaTp[0:80, :])
                nc.tensor.matmul(Eps[:], At_t[:], etail[:], start=False,
                                 stop=True)
                # E back to token-major
                nc.scalar.copy(e81[:], Eps[:])
                Etp = pp.tile([128, 128], F32, tag="zTp")
                nc.tensor.transpose(Etp[:, 0:81], e81[:], id_t[0:81, 0:81])
                nc.scalar.copy(Etok[:], Etp[:, 0:81])
                # --- s* argmax, d* = 80 - s* ---
                nc.vector.max(mx8[:], Etok[:])
                nc.vector.max_index(mi8[:], mx8[:], Etok[:])
                nc.vector.tensor_copy(sf[:], mi8[:, 0:1])
                nc.vector.tensor_scalar_mul(df[:], sf[:], -1.0)
                nc.vector.tensor_scalar_add(df[:], df[:], 80.0)
                # --- yhat embed: idx = iota160 + s* ---
                nc.vector.scalar_tensor_tensor(idxf[:, 0:160], i160_t[:],
                                               sf[:, 0:1], i160_t[:],
                                               Op.add, Op.bypass)
                gather_extract(yap[:], 240, 80, yhat, g1280)
                nc.vector.tensor_copy(idxi[:, 0:80], idxf[:, 80:160])
                nc.gpsimd.ap_gather(g1280[:], yap[:], idxi[:, 0:80],
                                    channels=128, num_elems=240, d=1,
                                    num_idxs=1280)
                gv2 = g1280[:].rearrange("p (j k) -> p j k", k=16)
                for k2 in range(16):
                    nc.vector.copy_predicated(
                        yhat[:, 80:160],
                        M_t[:, k2:k2 + 1].to_broadcast((P, 80)),
                        gv2[:, :, k2])
                # --- h_selT = W_enc @ yhat^T (+ b_enc) ---
                yhTp = pp.tile([128, 128], F32, tag="zTp")
                nc.tensor.transpose(yhTp[:], yhat[:, 0:128], id_t[:])
                nc.scalar.copy(yhT0[:], yhTp[:])
                yhTp2 = pp.tile([128, 128], F32, tag="zTp")
                nc.tensor.transpose(yhTp2[0:32, :], yhat[:, 128:160], id_t[:])
                nc.scalar.copy(yhT1[:], yhTp2[0:32, :])
                for hc in range(4):
                    Hp = pp.tile([128, 128], F32, tag="Hp")
                    nc.tensor.matmul(Hp[:], We_t[:, hc * 128:(hc + 1) * 128],
                                     yhT0[:], start=True, stop=False)
                    nc.tensor.matmul(Hp[:],
                                     We_t[0:32, 512 + hc * 128:512 + (hc + 1) * 128],
                                     yhT1[:], start=False, stop=True)
                    nc.scalar.copy(hsT[:, hc * 128:(hc + 1) * 128], Hp[:])
                    nc.vector.tensor_scalar_add(hsT[:, hc * 128:(hc + 1) * 128],
                                                hsT[:, hc * 128:(hc + 1) * 128],
                                                be_t[:, hc:hc + 1])
                # --- x_extT = W_src @ h_selT (+ b_src) ---
                for oc in range(2):
                    ow = 128 if oc == 0 else 32
                    Xp = pp.tile([128, 128], F32, tag="Hp")
                    for hc in range(4):
                        nc.tensor.matmul(
                            Xp[0:ow, :],
                            Ws_t[:, hc * 160 + oc * 128: hc * 160 + oc * 128 + ow],
                            hsT[:, hc * 128:(hc + 1) * 128],
                            start=(hc == 0), stop=(hc == 3))
                    dst = xeT0 if oc == 0 else xeT1
                    nc.scalar.copy(dst[:], Xp[0:ow, :])
                    nc.vector.tensor_scalar_add(dst[:], dst[:],
                                                bs_t[0:ow, oc:oc + 1])
                Xtp = pp.tile([128, 128], F32, tag="Hp")
                nc.tensor.transpose(Xtp[:], xeT0[:], id_t[:])
                nc.scalar.copy(xext[:, 0:128], Xtp[:])
                Xtp2 = pp.tile([128, 128], F32, tag="Hp")
                nc.tensor.transpose(Xtp2[:, 0:32], xeT1[:], id_t[0:32, 0:32])
                nc.scalar.copy(xext[:, 128:160], Xtp2[:, 0:32])
                # --- y_ele gather: idx = iota80 + d* ---
                nc.vector.scalar_tensor_tensor(idxf[:, 0:80], i80_t[:],
                                               df[:, 0:1], i80_t[:],
                                               Op.add, Op.bypass)
                gather_extract(xext[:], 160, 80, yele, g1280)
                # --- loss partial + state updates ---
                nc.vector.tensor_tensor(dtmp[:], yele[:], yres[:], Op.subtract)
                nc.vector.tensor_tensor(dtmp[:], dtmp[:], keep[:], Op.mult)
                nc.vector.tensor_tensor(et[:], dtmp[:], dtmp[:], Op.mult)
                nc.vector.tensor_reduce(lossp[:, it:it + 1], et[:],
                                        mybir.AxisListType.X, Op.add)
                nc.vector.tensor_tensor(yres[:], yres[:], yele[:], Op.subtract)

            nc.sync.dma_start(d_out[:], lossp[:])
    return nc


def kernel(x, y, W_enc, b_enc, W_src, b_src):
    import sys
    if '/opt/trn_rl_repo' not in sys.path:
        sys.path.insert(0, '/opt/trn_rl_repo')
    x = np.asarray(x, np.float32)
    y = np.asarray(y, np.float32)
    consts = _build_consts(W_enc, b_enc, W_src, b_src)

    if "nc" not in _cache:
        _cache["nc"] = _build_nc()
        _cache["nc"].finalize()
    nc = _cache["nc"]

    xt = x.reshape(NTOK, IDIM)
    yt = y.reshape(NTOK, ODIM)
    in_maps = []
    for c in range(NCORES):
        m = dict(consts)
        m["x"] = np.ascontiguousarray(xt[c * P:(c + 1) * P])
        m["y"] = np.ascontiguousarray(yt[c * P:(c + 1) * P])
        in_maps.append(m)

    from concourse.bass_utils import run_bass_kernel_spmd
    res = run_bass_kernel_spmd(nc, in_maps, list(range(NCORES)))
    parts = np.stack([r["losspart"] for r in res.results])
    keep_cnt = max(int((y != 0.0).sum()), 1)
    nums = parts[:, :, :THINK_ITER].sum(axis=(0, 1), dtype=np.float64)
    losses = (nums / keep_cnt).astype(np.float32)
    return np.float32(np.mean(losses))



# revision 6
# speedup vs baseline: 1.0398x; 1.0398x over previous
"""Trainium2 Bass kernel for nn_Net_17532056502451.

5 "think" iterations: shift-window cosine selector (159 shifts) + softmax
attention + scatter-back + conv-style encoder/decoder with energy argmax
(81 shifts), masked-MSE losses averaged.  Data-parallel: 1024 tokens over
8 cores, 128 tokens/core (one per SBUF partition), token-major.

v2 mappings per core:
- dot correlation: ONE bf16 tensor_tensor (2 elem/cyc) building all 12720
  products + bf16 tree adds (80->40->20->10->5) + fp32 tensor_reduce.
- sliding norms: Square + prefix-scan + strided diff (fp32).
- per-token dynamic windows: GPSIMD ap_gather (idx[p,j] = off_p + j, 16
  candidate lanes) + mask-mult + tensor_reduce diagonal extract (2 DVE ops
  instead of 16 predicated copies).
- energy: quadratic Gram form, all-bf16 PE pipeline: z built in one DVE op,
  transposed on PE (bf16, 1 cyc/row), PSUM->SBUF copies batched 4 chunks at
  a time, E matmuls flipped (z chunk stationary, Az moving) so E lands
  token-major in PSUM - no transpose-back.
- encoder: h-major H matmuls with b_enc folded in via a ones row;
  decoder: X matmuls flipped token-major with b_src via a k=1 matmul -
  no back-transposes.
- softmax exp + denominator in one scalar-engine op (accum_out); loss
  sum-of-squares via scalar Square + accum_out.
"""
import numpy as np

IDIM = 80
ODIM = 80
HDIM = 512
THINK_ITER = 5
TEMPER = 0.7
B, T = 4, 256
NTOK = B * T
P = 128
NCORES = 8
S1 = 159
S2 = 81
NFEAT = 80 * 80
NCHUNK = NFEAT // 128   # 50

_cache = {}


def _bf16(a):
    import ml_dtypes
    return np.asarray(a, dtype=ml_dtypes.bfloat16)


def _build_consts(W_enc, b_enc, W_src, b_src):
    W_enc = np.asarray(W_enc, np.float32)
    b_enc = np.asarray(b_enc, np.float32)
    W_src = np.asarray(W_src, np.float32)
    b_src = np.asarray(b_src, np.float32)
    C = (W_enc.T @ W_enc).astype(np.float32)
    q = (W_enc.T @ b_enc).astype(np.float32)
    bb = np.float32(b_enc @ b_enc)
    # E[t,s] = sum_{d,i} Az[s, 80d+i] * ye_i ye_{i+d} + sum_i 2 q[dd+i] ye_i
    #          + bb,  dd = 80 - s
    Az = np.zeros((S2, NFEAT), np.float32)
    Al = np.zeros((S2, 81), np.float32)
    for s in range(S2):
        dd = 80 - s
        blk = C[dd:dd + 80, dd:dd + 80]
        for d in range(80):
            diag = np.diagonal(blk, offset=d).copy()
            Az[s, d * 80: d * 80 + (80 - d)] = (2.0 if d > 0 else 1.0) * diag
        Al[s, :80] = 2.0 * q[dd:dd + 80]
        Al[s, 80] = bb
    Az_cat = np.ascontiguousarray(Az.T)               # (6400, 81): pi-major
    Atail = np.ascontiguousarray(Al.T)                # (81, 81)
    W_encT = np.ascontiguousarray(W_enc.T)            # (160, 512)
    We1 = W_encT[0:128]                               # (128, 512)
    We2 = np.zeros((33, 512), np.float32)             # 32 c's + bias row
    We2[0:32] = W_encT[128:160]
    We2[32] = b_enc
    W_srcT = np.ascontiguousarray(W_src.T)            # (512, 160)
    M16 = np.zeros((P, 16), np.float32)
    for p in range(P):
        M16[p, p % 16] = 1.0
    iota80 = np.broadcast_to(np.arange(80, dtype=np.float32), (P, 80)).copy()
    iota160 = np.broadcast_to(np.arange(160, dtype=np.float32), (P, 160)).copy()
    ident = np.eye(128, dtype=np.float32)
    return dict(Az=_bf16(Az_cat), Atail=_bf16(Atail),
                We1=_bf16(We1), We2=_bf16(We2), Ws=_bf16(W_srcT),
                bsrc=_bf16(b_src.reshape(1, 160)),
                ones1=_bf16(np.ones((1, 128), np.float32)),
                ident=_bf16(ident), M16=M16,
                iota80=iota80, iota160=iota160)


def _build_nc():
    import concourse.bass as bass
    import concourse.bacc as bacc
    import concourse.mybir as mybir
    from concourse.tile import TileContext

    F32 = mybir.dt.float32
    BF16 = mybir.dt.bfloat16
    I16 = mybir.dt.int16
    U32 = mybir.dt.uint32
    Op = mybir.AluOpType
    AF = mybir.ActivationFunctionType
    AX = mybir.AxisListType

    nc = bacc.Bacc()
    d_x = nc.declare_dram_parameter("x", [P, 80], F32, isOutput=False)
    d_y = nc.declare_dram_parameter("y", [P, 80], F32, isOutput=False)
    d_A = nc.declare_dram_parameter("Az", [NFEAT, 81], BF16, isOutput=False)
    d_At = nc.declare_dram_parameter("Atail", [81, 81], BF16, isOutput=False)
    d_We1 = nc.declare_dram_parameter("We1", [128, 512], BF16, isOutput=False)
    d_We2 = nc.declare_dram_parameter("We2", [33, 512], BF16, isOutput=False)
    d_Ws = nc.declare_dram_parameter("Ws", [512, 160], BF16, isOutput=False)
    d_bs = nc.declare_dram_parameter("bsrc", [1, 160], BF16, isOutput=False)
    d_on = nc.declare_dram_parameter("ones1", [1, 128], BF16, isOutput=False)
    d_id = nc.declare_dram_parameter("ident", [128, 128], BF16, isOutput=False)
    d_M = nc.declare_dram_parameter("M16", [P, 16], F32, isOutput=False)
    d_i80 = nc.declare_dram_parameter("iota80", [P, 80], F32, isOutput=False)
    d_i160 = nc.declare_dram_parameter("iota160", [P, 160], F32, isOutput=False)
    d_out = nc.declare_dram_parameter("losspart", [P, 8], F32, isOutput=True)

    with TileContext(nc) as tc:
        with (
            tc.tile_pool(name="const", bufs=1) as cpool,
            tc.tile_pool(name="work", bufs=1) as pool,
            tc.tile_pool(name="ps_rot", bufs=2, space="PSUM") as pp,
            tc.tile_pool(name="ps_e", bufs=1, space="PSUM") as ppe,
            tc.tile_pool(name="ps_h", bufs=1, space="PSUM") as pph,
            tc.tile_pool(name="ps_x", bufs=1, space="PSUM") as ppx,
        ):
            # ---- constants ----
            A_t = cpool.tile([P, NCHUNK * 81], BF16, tag="A")
            for k in range(NCHUNK):
                nc.sync.dma_start(A_t[:, k * 81:(k + 1) * 81],
                                  d_A[k * 128:(k + 1) * 128, :])
            At_t = cpool.tile([81, 81], BF16, tag="At")
            nc.sync.dma_start(At_t[:], d_At[:])
            We1_t = cpool.tile([128, 512], BF16, tag="We1")
            nc.sync.dma_start(We1_t[:], d_We1[:])
            We2_t = cpool.tile([33, 512], BF16, tag="We2")
            nc.sync.dma_start(We2_t[:], d_We2[:])
            Ws_t = cpool.tile([P, 4 * 160], BF16, tag="Ws")
            for k in range(4):
                nc.sync.dma_start(Ws_t[:, k * 160:(k + 1) * 160],
                                  d_Ws[k * 128:(k + 1) * 128, :])
            bs_t = cpool.tile([1, 160], BF16, tag="bs")
            nc.sync.dma_start(bs_t[:], d_bs[:])
            on_t = cpool.tile([1, 128], BF16, tag="on")
            nc.sync.dma_start(on_t[:], d_on[:])
            id_t = cpool.tile([128, 128], BF16, tag="id")
            nc.sync.dma_start(id_t[:], d_id[:])
            M_t = cpool.tile([P, 16], F32, tag="M")
            nc.sync.dma_start(M_t[:], d_M[:])
            i80_t = cpool.tile([P, 80], F32, tag="i80")
            nc.sync.dma_start(i80_t[:], d_i80[:])
            i160_t = cpool.tile([P, 160], F32, tag="i160")
            nc.sync.dma_start(i160_t[:], d_i160[:])

            # ---- state ----
            xpad = pool.tile([P, 238], F32, tag="xpad")
            yres = pool.tile([P, 80], F32, tag="yres")
            keep = pool.tile([P, 80], F32, tag="keep")
            yap = pool.tile([P, 240], F32, tag="yap")
            lossp = pool.tile([P, 8], F32, tag="lossp")
            xpadh = pool.tile([P, 238], BF16, tag="xpadh")
            yresh = pool.tile([P, 80], BF16, tag="yresh")
            yah = pool.tile([P, 160], BF16, tag="yah")
            nc.vector.memset(xpad[:], 0.0)
            nc.vector.memset(yap[:], 0.0)
            nc.vector.memset(lossp[:], 0.0)
            nc.sync.dma_start(xpad[:, 79:159], d_x[:])
            nc.sync.dma_start(yres[:], d_y[:])
            nc.vector.tensor_scalar(keep[:], yres[:], 0.0, None, Op.not_equal)

            # dot workspace
            dtp = pool.tile([P, S1 * 80], BF16, tag="dtp")
            t40 = pool.tile([P, S1 * 40], BF16, tag="t40")
            t20 = pool.tile([P, S1 * 20], BF16, tag="t20")
            t10 = pool.tile([P, S1 * 10], BF16, tag="t10")
            t5 = pool.tile([P, S1 * 5], BF16, tag="t5")
            dot = pool.tile([P, S1], F32, tag="dot")
            adot = pool.tile([P, S1], F32, tag="adot")
            gsel = pool.tile([P, S1], F32, tag="gsel")
            sqx = pool.tile([P, 239], F32, tag="sqx")
            cs = pool.tile([P, 239], F32, tag="cs")
            nsq = pool.tile([P, S1], F32, tag="nsq")
            rnsq = pool.tile([P, S1], F32, tag="rnsq")
            zero1 = pool.tile([P, 1], F32, tag="zero1")
            nc.vector.memset(zero1[:], 0.0)
            nc.vector.memset(sqx[:, 0:1], 0.0)

            mx8 = pool.tile([P, 8], F32, tag="mx8")
            mi8 = pool.tile([P, 8], U32, tag="mi8")
            thf = pool.tile([P, 1], F32, tag="thf")
            thx = pool.tile([P, 1], F32, tag="thx")
            sf = pool.tile([P, 1], F32, tag="sf")
            df = pool.tile([P, 1], F32, tag="df")
            idxA = pool.tile([P, 80], I16, tag="idxA")
            idxB = pool.tile([P, 80], I16, tag="idxB")
            idxC = pool.tile([P, 160], I16, tag="idxC")
            idxD = pool.tile([P, 80], I16, tag="idxD")
            idxf = pool.tile([P, 160], F32, tag="idxf")
            gvA = pool.tile([P, 1280], F32, tag="gvA")
            gvB = pool.tile([P, 1280], F32, tag="gvB")
            gvC = pool.tile([P, 2560], F32, tag="gvC")
            gvD = pool.tile([P, 1280], F32, tag="gvD")
            gvm = pool.tile([P, 2560], F32, tag="gvm")

            yal = pool.tile([P, 80], F32, tag="yal")
            xele = pool.tile([P, 80], F32, tag="xele")
            yele = pool.tile([P, 80], F32, tag="yele")
            yhatf = pool.tile([P, 160], F32, tag="yhatf")
            yhath = pool.tile([P, 160], BF16, tag="yhath")
            zt = pool.tile([P, 80], F32, tag="zt")
            et = pool.tile([P, 80], F32, tag="et")
            ssum = pool.tile([P, 1], F32, tag="ssum")
            rsum = pool.tile([P, 1], F32, tag="rsum")
            nzm = pool.tile([P, 1], F32, tag="nzm")

            zfeat = pool.tile([P, NFEAT], BF16, tag="zfeat")
            zsb = pool.tile([P, NFEAT], BF16, tag="zsb")
            etail = pool.tile([81, 128], BF16, tag="etail")
            nc.sync.dma_start(etail[80:81, :], d_on[:])
            Etok = pool.tile([P, S2], F32, tag="Etok")
            yh0 = pool.tile([128, 128], BF16, tag="yh0")
            yh1 = pool.tile([33, 128], BF16, tag="yh1")
            nc.sync.dma_start(yh1[32:33, :], d_on[:])
            hsb = pool.tile([128, 512], BF16, tag="hsb")
            xext = pool.tile([P, 160], F32, tag="xext")
            dtmp = pool.tile([P, 80], F32, tag="dtmp")
            dsq = pool.tile([P, 80], F32, tag="dsq")

            # initial bf16 shadows + sliding-norm prep for iter 0
            nc.scalar.copy(xpadh[:], xpad[:])
            nc.scalar.copy(yresh[:], yres[:])
            nc.scalar.activation(sqx[:, 1:239], xpad[:], AF.Square)
            nc.vector.tensor_tensor_scan(cs[:], sqx[:],
                                         zero1[:].to_broadcast((P, 239)),
                                         0.0, Op.add, Op.bypass)
            nc.vector.tensor_tensor(nsq[:], cs[:, 80:239], cs[:, 0:159],
                                    Op.subtract)

            def extract(gv, width, out_tile):
                """out[p, j] = gv[p, 16*j + (p%16)] via mask-mult + reduce."""
                gvv = gv[:, 0:16 * width].rearrange("p (j k) -> p j k", k=16)
                gmv = gvm[:, 0:16 * width].rearrange("p (j k) -> p j k", k=16)
                nc.vector.tensor_tensor(
                    gmv, gvv, M_t[:].unsqueeze(1).to_broadcast((P, width, 16)),
                    Op.mult)
                nc.vector.tensor_reduce(out_tile[:, 0:width], gmv, AX.X, Op.add)

            for it in range(THINK_ITER):
                # --- dot[p,s] = sum_c xpad[p,s+c]*yres[p,c]: bf16 tree ---
                in0 = bass.AP(xpadh.tensor, xpadh.offset,
                              [list(xpadh.ap[0]), [1, S1], [1, 80]])
                in1 = bass.AP(yresh.tensor, yresh.offset,
                              [list(yresh.ap[0]), [0, S1], [1, 80]])
                dview = dtp[:].rearrange("p (s c) -> p s c", c=80)
                nc.vector.tensor_tensor(dview, in0, in1, Op.mult)
                for (src, dst, w) in ((dtp, t40, 40), (t40, t20, 20),
                                      (t20, t10, 10), (t10, t5, 5)):
                    sv = src[:].rearrange("p (s c) -> p s c", c=2 * w)
                    dv = dst[:].rearrange("p (s c) -> p s c", c=w)
                    nc.vector.tensor_tensor(dv, sv[:, :, 0:w], sv[:, :, w:2 * w],
                                            Op.add)
                nc.vector.tensor_reduce(
                    dot[:], t5[:].rearrange("p (s c) -> p s c", c=5), AX.X,
                    Op.add)
                # --- theta = argmax dot*|dot|/nsq ---
                nc.vector.tensor_scalar_max(rnsq[:], nsq[:], 1e-30)
                nc.vector.reciprocal(rnsq[:], rnsq[:])
                nc.scalar.activation(adot[:], dot[:], AF.Abs)
                nc.vector.tensor_tensor(gsel[:], dot[:], adot[:], Op.mult)
                nc.vector.tensor_tensor(gsel[:], gsel[:], rnsq[:], Op.mult)
                nc.vector.max(mx8[:], gsel[:])
                nc.vector.max_index(mi8[:], mx8[:], gsel[:])
                nc.vector.tensor_copy(thf[:], mi8[:, 0:1])
                # --- y_align gather ---
                nc.vector.scalar_tensor_tensor(idxf[:, 0:80], i80_t[:],
                                               thf[:, 0:1], i80_t[:],
                                               Op.add, Op.bypass)
                nc.vector.tensor_copy(idxA[:], idxf[:, 0:80])
                nc.gpsimd.ap_gather(gvA[:], xpad[:], idxA[:], channels=128,
                                    num_elems=238, d=1, num_idxs=1280)
                extract(gvA, 80, yal)
                # --- softmax attention -> y_att in yap[:, 80:160] ---
                nc.vector.tensor_tensor(zt[:], yal[:], yres[:], Op.mult)
                nc.vector.max(mx8[:], zt[:])
                nc.vector.tensor_scalar_mul(nzm[:], mx8[:, 0:1], -1.0 / TEMPER)
                nc.scalar.activation(et[:], zt[:], AF.Exp, bias=nzm[:, 0:1],
                                     scale=1.0 / TEMPER,
                                     accum_out=ssum[:, 0:1])
                nc.vector.reciprocal(rsum[:], ssum[:])
                nc.vector.scalar_tensor_tensor(yap[:, 80:160], et[:],
                                               rsum[:, 0:1], yal[:],
                                               Op.mult, Op.mult)
                nc.scalar.copy(yah[:], yap[:, 80:240])
                # --- z features z[p, 80d+i] = ye[i]*ye[i+d] (bf16, one op) ---
                zin0 = bass.AP(yah.tensor, yah.offset,
                               [list(yah.ap[0]), [0, 80], [1, 80]])
                zin1 = bass.AP(yah.tensor, yah.offset,
                               [list(yah.ap[0]), [1, 80], [1, 80]])
                zv = zfeat[:].rearrange("p (d i) -> p d i", i=80)
                nc.vector.tensor_tensor(zv, zin0, zin1, Op.mult)
                # --- tail features: etail = [ya^T (80); ones] ---
                yaTp = pp.tile([128, 128], BF16, tag="tp")
                nc.tensor.transpose(yaTp[0:80, :], yah[:, 0:80], id_t[:])
                nc.scalar.copy(etail[0:80, :], yaTp[0:80, :])
                # --- E accumulation: T (PE) -> copy (S) -> flipped MM (PE) ---
                Eps = ppe.tile([P, S2], mybir.dt.float32, tag="Eps")
                nbatch = (NCHUNK + 3) // 4  # 13 batches of <=4 chunks
                zpss = [None] * nbatch
                for bch in range(nbatch + 1):
                    if bch < nbatch:
                        k0 = bch * 4
                        kn = min(4, NCHUNK - k0)
                        zps = pp.tile([128, 512], BF16, tag="zps")
                        zpss[bch] = (zps, k0, kn)
                        for q in range(kn):
                            k = k0 + q
                            nc.tensor.transpose(
                                zps[:, q * 128:(q + 1) * 128],
                                zfeat[:, k * 128:(k + 1) * 128], id_t[:])
                        nc.scalar.copy(zsb[:, k0 * 128:(k0 + kn) * 128],
                                       zps[:, 0:kn * 128])
                    j = bch - 1
                    if 0 <= j < nbatch:
                        zps, k0, kn = zpss[j]
                        for q in range(kn):
                            k = k0 + q
                            nc.tensor.matmul(
                                Eps[:], zsb[:, k * 128:(k + 1) * 128],
                                A_t[:, k * 81:(k + 1) * 81],
                                start=(k == 0), stop=False)
                nc.tensor.matmul(Eps[:], etail[:], At_t[:], start=False,
                                 stop=True)
                # --- x_ele gather (overlaps E pipe): idx = 159 - theta + j ---
                nc.vector.tensor_scalar(thx[:], thf[:], -1.0, 159.0,
                                        Op.mult, Op.add)
                nc.vector.scalar_tensor_tensor(idxf[:, 0:80], i80_t[:],
                                               thx[:, 0:1], i80_t[:],
                                               Op.add, Op.bypass)
                nc.vector.tensor_copy(idxB[:], idxf[:, 0:80])
                nc.gpsimd.ap_gather(gvB[:], yap[:], idxB[:], channels=128,
                                    num_elems=240, d=1, num_idxs=1280)
                extract(gvB, 80, xele)
                nc.vector.tensor_tensor(xpad[:, 79:159], xpad[:, 79:159],
                                        xele[:], Op.subtract)
                # next-iter prep (overlaps E pipe)
                nc.scalar.copy(xpadh[:], xpad[:])
                nc.scalar.activation(sqx[:, 1:239], xpad[:], AF.Square)
                nc.vector.tensor_tensor_scan(cs[:], sqx[:],
                                             zero1[:].to_broadcast((P, 239)),
                                             0.0, Op.add, Op.bypass)
                nc.vector.tensor_tensor(nsq[:], cs[:, 80:239], cs[:, 0:159],
                                        Op.subtract)
                # --- s* = argmax E (token-major PSUM -> SBUF), d* = 80-s* ---
                nc.scalar.copy(Etok[:], Eps[:])
                nc.vector.max(mx8[:], Etok[:])
                nc.vector.max_index(mi8[:], mx8[:], Etok[:])
                nc.vector.tensor_copy(sf[:], mi8[:, 0:1])
                nc.vector.tensor_scalar(df[:], sf[:], -1.0, 80.0,
                                        Op.mult, Op.add)
                # --- yhat gather: idx = s* + j, j in [0,160) ---
                nc.vector.scalar_tensor_tensor(idxf[:], i160_t[:],
                                               sf[:, 0:1], i160_t[:],
                                               Op.add, Op.bypass)
                nc.vector.tensor_copy(idxC[:], idxf[:])
                nc.gpsimd.ap_gather(gvC[:], yap[:], idxC[:], channels=128,
                                    num_elems=240, d=1, num_idxs=2560)
                extract(gvC, 160, yhatf)
                nc.scalar.copy(yhath[:], yhatf[:])
                # --- H = W_enc @ yhat^T + b_enc (h-major, bias folded) ---
                yh0p = pp.tile([128, 128], BF16, tag="tp")
                nc.tensor.transpose(yh0p[:], yhath[:, 0:128], id_t[:])
                nc.scalar.copy(yh0[:], yh0p[:])
                yh1p = pp.tile([128, 128], BF16, tag="tp")
                nc.tensor.transpose(yh1p[0:32, :], yhath[:, 128:160], id_t[:])
                nc.scalar.copy(yh1[0:32, :], yh1p[0:32, :])
                Hps = pph.tile([128, 512], mybir.dt.float32, tag="Hps")
                for hc in range(4):
                    r = slice(hc * 128, (hc + 1) * 128)
                    nc.tensor.matmul(Hps[:, r], We1_t[:, r], yh0[:],
                                     start=True, stop=False)
                    nc.tensor.matmul(Hps[:, r], We2_t[:, r], yh1[:],
                                     start=False, stop=True)
                nc.scalar.copy(hsb[:], Hps[:])
                # --- X = W_src @ h + b_src (token-major via flip) ---
                Xps = ppx.tile([128, 160], mybir.dt.float32, tag="Xps")
                for hc in range(4):
                    nc.tensor.matmul(Xps[:], hsb[:, hc * 128:(hc + 1) * 128],
                                     Ws_t[:, hc * 160:(hc + 1) * 160],
                                     start=(hc == 0), stop=False)
                nc.tensor.matmul(Xps[:], on_t[:], bs_t[:], start=False,
                                 stop=True)
                nc.scalar.copy(xext[:], Xps[:])
                # --- y_ele gather: idx = d* + j ---
                nc.vector.scalar_tensor_tensor(idxf[:, 0:80], i80_t[:],
                                               df[:, 0:1], i80_t[:],
                                               Op.add, Op.bypass)
                nc.vector.tensor_copy(idxD[:], idxf[:, 0:80])
                nc.gpsimd.ap_gather(gvD[:], xext[:], idxD[:], channels=128,
                                    num_elems=160, d=1, num_idxs=1280)
                extract(gvD, 80, yele)
                # --- loss partial + state updates ---
                nc.vector.tensor_tensor(dtmp[:], yele[:], yres[:], Op.subtract)
                nc.vector.tensor_tensor(dtmp[:], dtmp[:], keep[:], Op.mult)
                nc.scalar.activation(dsq[:], dtmp[:], AF.Square,
                                     accum_out=lossp[:, it:it + 1])
                nc.vector.tensor_tensor(yres[:], yres[:], yele[:], Op.subtract)
                nc.scalar.copy(yresh[:], yres[:])

            nc.sync.dma_start(d_out[:], lossp[:])
    return nc


def kernel(x, y, W_enc, b_enc, W_src, b_src):
    import sys
    if '/opt/trn_rl_repo' not in sys.path:
        sys.path.insert(0, '/opt/trn_rl_repo')
    x = np.asarray(x, np.float32)
    y = np.asarray(y, np.float32)
    consts = _build_consts(W_enc, b_enc, W_src, b_src)

    if "nc" not in _cache:
        _cache["nc"] = _build_nc()
        _cache["nc"].finalize()
    nc = _cache["nc"]

    xt = x.reshape(NTOK, IDIM)
    yt = y.reshape(NTOK, ODIM)
    in_maps = []
    for c in range(NCORES):
        m = dict(consts)
        m["x"] = np.ascontiguousarray(xt[c * P:(c + 1) * P])
        m["y"] = np.ascontiguousarray(yt[c * P:(c + 1) * P])
        in_maps.append(m)

    from concourse.bass_utils import run_bass_kernel_spmd
    res = run_bass_kernel_spmd(nc, in_maps, list(range(NCORES)))
    parts = np.stack([r["losspart"] for r in res.results])
    keep_cnt = max(int((y != 0.0).sum()), 1)
    nums = parts[:, :, :THINK_ITER].sum(axis=(0, 1), dtype=np.float64)
    losses = (nums / keep_cnt).astype(np.float32)
    return np.float32(np.mean(losses))


# revision 20
# speedup vs baseline: 4.5922x; 4.4166x over previous
"""Trainium2 Bass kernel for nn_Net_17532056502451.

5 "think" iterations: shift-window cosine selector (159 shifts) + softmax
attention + scatter-back + conv-style encoder/decoder with energy argmax
(81 shifts), masked-MSE losses averaged.  Data-parallel: 1024 tokens over
8 cores, 128 tokens/core (one per SBUF partition), token-major.

v2 mappings per core:
- dot correlation: ONE bf16 tensor_tensor (2 elem/cyc) building all 12720
  products + bf16 tree adds (80->40->20->10->5) + fp32 tensor_reduce.
- sliding norms: Square + prefix-scan + strided diff (fp32).
- per-token dynamic windows: GPSIMD ap_gather (idx[p,j] = off_p + j, 16
  candidate lanes) + mask-mult + tensor_reduce diagonal extract (2 DVE ops
  instead of 16 predicated copies).
- energy: quadratic Gram form, all-bf16 PE pipeline: z built in one DVE op,
  transposed on PE (bf16, 1 cyc/row), PSUM->SBUF copies batched 4 chunks at
  a time, E matmuls flipped (z chunk stationary, Az moving) so E lands
  token-major in PSUM - no transpose-back.
- encoder: h-major H matmuls with b_enc folded in via a ones row;
  decoder: X matmuls flipped token-major with b_src via a k=1 matmul -
  no back-transposes.
- softmax exp + denominator in one scalar-engine op (accum_out); loss
  sum-of-squares via scalar Square + accum_out.
"""
import numpy as np

IDIM = 80
ODIM = 80
HDIM = 512
THINK_ITER = 5
TEMPER = 0.7
B, T = 4, 256
NTOK = B * T
P = 128
NCORES = 8
S1 = 159
S2 = 81
NFEAT = 80 * 80
NCHUNK = NFEAT // 128   # 50

_cache = {}


def _bf16(a):
    import ml_dtypes
    return np.asarray(a, dtype=ml_dtypes.bfloat16)


def _build_consts(W_enc, b_enc, W_src, b_src):
    W_enc = np.asarray(W_enc, np.float32)
    b_enc = np.asarray(b_enc, np.float32)
    W_src = np.asarray(W_src, np.float32)
    b_src = np.asarray(b_src, np.float32)
    C = (W_enc.T @ W_enc).astype(np.float32)
    q = (W_enc.T @ b_enc).astype(np.float32)
    bb = np.float32(b_enc @ b_enc)
    # E[t,s] = sum_{d,i} Az[s, 80d+i] * ye_i ye_{i+d} + sum_i 2 q[dd+i] ye_i
    #          + bb,  dd = 80 - s
    Az = np.zeros((S2, NFEAT), np.float32)
    Al = np.zeros((S2, 81), np.float32)
    for s in range(S2):
        dd = 80 - s
        blk = C[dd:dd + 80, dd:dd + 80]
        for d in range(80):
            diag = np.diagonal(blk, offset=d).copy()
            Az[s, d * 80: d * 80 + (80 - d)] = (2.0 if d > 0 else 1.0) * diag
        Al[s, :80] = 2.0 * q[dd:dd + 80]
        Al[s, 80] = bb
    Az_cat = np.ascontiguousarray(Az.T)               # (6400, 81): pi-major
    Atail = np.ascontiguousarray(Al.T)                # (81, 81)
    W_encT = np.ascontiguousarray(W_enc.T)            # (160, 512)
    We1 = W_encT[0:128]                               # (128, 512)
    We2 = np.zeros((33, 512), np.float32)             # 32 c's + bias row
    We2[0:32] = W_encT[128:160]
    We2[32] = b_enc
    W_srcT = np.ascontiguousarray(W_src.T)            # (512, 160)
    iota8 = np.broadcast_to(np.arange(8, dtype=np.int32), (P, 8)).copy()
    ident = np.eye(128, dtype=np.float32)
    return dict(Az=_bf16(Az_cat), Atail=_bf16(Atail),
                We1=_bf16(We1), We2=_bf16(We2), Ws=_bf16(W_srcT),
                bsrc=_bf16(b_src.reshape(1, 160)),
                ones1=_bf16(np.ones((1, 128), np.float32)),
                ident=_bf16(ident), iota8=iota8)


def _build_nc():
    import concourse.bass as bass
    import concourse.bacc as bacc
    import concourse.mybir as mybir
    from concourse.tile import TileContext

    F32 = mybir.dt.float32
    BF16 = mybir.dt.bfloat16
    I16 = mybir.dt.int16
    U32 = mybir.dt.uint32
    Op = mybir.AluOpType
    AF = mybir.ActivationFunctionType
    AX = mybir.AxisListType

    nc = bacc.Bacc()
    d_x = nc.declare_dram_parameter("x", [P, 80], F32, isOutput=False)
    d_y = nc.declare_dram_parameter("y", [P, 80], F32, isOutput=False)
    d_A = nc.declare_dram_parameter("Az", [NFEAT, 81], BF16, isOutput=False)
    d_At = nc.declare_dram_parameter("Atail", [81, 81], BF16, isOutput=False)
    d_We1 = nc.declare_dram_parameter("We1", [128, 512], BF16, isOutput=False)
    d_We2 = nc.declare_dram_parameter("We2", [33, 512], BF16, isOutput=False)
    d_Ws = nc.declare_dram_parameter("Ws", [512, 160], BF16, isOutput=False)
    d_bs = nc.declare_dram_parameter("bsrc", [1, 160], BF16, isOutput=False)
    d_on = nc.declare_dram_parameter("ones1", [1, 128], BF16, isOutput=False)
    d_id = nc.declare_dram_parameter("ident", [128, 128], BF16, isOutput=False)
    d_i8 = nc.declare_dram_parameter("iota8", [P, 8], mybir.dt.int32,
                                     isOutput=False)
    d_out = nc.declare_dram_parameter("losspart", [P, 8], F32, isOutput=True)

    with TileContext(nc) as tc:
        with (
            tc.tile_pool(name="const", bufs=1) as cpool,
            tc.tile_pool(name="work", bufs=1) as pool,
            tc.tile_pool(name="ps_rot", bufs=2, space="PSUM") as pp,
            tc.tile_pool(name="ps_e", bufs=1, space="PSUM") as ppe,
            tc.tile_pool(name="ps_h", bufs=1, space="PSUM") as pph,
            tc.tile_pool(name="ps_x", bufs=1, space="PSUM") as ppx,
        ):
            # ---- constants ----
            A_t = cpool.tile([P, NCHUNK * 81], BF16, tag="A")
            for k in range(NCHUNK):
                nc.sync.dma_start(A_t[:, k * 81:(k + 1) * 81],
                                  d_A[k * 128:(k + 1) * 128, :])
            At_t = cpool.tile([81, 81], BF16, tag="At")
            nc.sync.dma_start(At_t[:], d_At[:])
            We1_t = cpool.tile([128, 512], BF16, tag="We1")
            nc.sync.dma_start(We1_t[:], d_We1[:])
            We2_t = cpool.tile([33, 512], BF16, tag="We2")
            nc.sync.dma_start(We2_t[:], d_We2[:])
            Ws_t = cpool.tile([P, 4 * 160], BF16, tag="Ws")
            for k in range(4):
                nc.sync.dma_start(Ws_t[:, k * 160:(k + 1) * 160],
                                  d_Ws[k * 128:(k + 1) * 128, :])
            bs_t = cpool.tile([1, 160], BF16, tag="bs")
            nc.sync.dma_start(bs_t[:], d_bs[:])
            on_t = cpool.tile([1, 128], BF16, tag="on")
            nc.sync.dma_start(on_t[:], d_on[:])
            id_t = cpool.tile([128, 128], BF16, tag="id")
            nc.sync.dma_start(id_t[:], d_id[:])
            i8_t = cpool.tile([P, 8], mybir.dt.int32, tag="i8")
            nc.sync.dma_start(i8_t[:], d_i8[:])

            # ---- state ----
            xpad = pool.tile([P, 238], F32, tag="xpad")
            yres = pool.tile([P, 80], F32, tag="yres")
            keep = pool.tile([P, 80], F32, tag="keep")
            yap = pool.tile([P, 240], F32, tag="yap")
            lossp = pool.tile([P, 8], F32, tag="lossp")
            xpadh = pool.tile([P, 238], BF16, tag="xpadh")
            yresh = pool.tile([P, 80], BF16, tag="yresh")
            yah = pool.tile([P, 160], BF16, tag="yah")
            nc.vector.memset(xpad[:], 0.0)
            nc.vector.memset(yap[:], 0.0)
            nc.vector.memset(lossp[:], 0.0)
            nc.sync.dma_start(xpad[:, 79:159], d_x[:])
            nc.sync.dma_start(yres[:], d_y[:])
            nc.vector.tensor_scalar(keep[:], yres[:], 0.0, None, Op.not_equal)

            # dot workspace
            dtp = pool.tile([P, S1 * 80], BF16, tag="dtp")
            t40 = pool.tile([P, S1 * 40], BF16, tag="t40")
            t20 = pool.tile([P, S1 * 20], BF16, tag="t20")
            t10 = pool.tile([P, S1 * 10], BF16, tag="t10")
            t5 = pool.tile([P, S1 * 5], BF16, tag="t5")
            dot = pool.tile([P, S1], F32, tag="dot")
            adot = pool.tile([P, S1], F32, tag="adot")
            gsel = pool.tile([P, S1], F32, tag="gsel")
            sqx = pool.tile([P, 239], F32, tag="sqx")
            cs = pool.tile([P, 239], F32, tag="cs")
            nsq = pool.tile([P, S1], F32, tag="nsq")
            rnsq = pool.tile([P, S1], F32, tag="rnsq")
            zero1 = pool.tile([P, 1], F32, tag="zero1")
            nc.vector.memset(zero1[:], 0.0)
            nc.vector.memset(sqx[:, 0:1], 0.0)

            I32 = mybir.dt.int32
            mx8 = pool.tile([P, 8], F32, tag="mx8")
            mi8 = pool.tile([P, 8], U32, tag="mi8")
            thf = pool.tile([P, 1], F32, tag="thf")
            thxf = pool.tile([P, 1], F32, tag="thxf")
            sff = pool.tile([P, 1], F32, tag="sff")
            dff = pool.tile([P, 1], F32, tag="dff")
            offI = pool.tile([P, 1], I32, tag="offI")
            bits = pool.tile([P, 8], I32, tag="bits")
            onesI = pool.tile([P, 8], I32, tag="onesI")
            nc.vector.memset(onesI[:], 1)
            wb1 = pool.tile([P, 240], F32, tag="wb1")
            wb2 = pool.tile([P, 240], F32, tag="wb2")
            yhath = pool.tile([P, 160], BF16, tag="yhath")
            zt = pool.tile([P, 80], F32, tag="zt")
            et = pool.tile([P, 80], F32, tag="et")
            ssum = pool.tile([P, 1], F32, tag="ssum")
            rsum = pool.tile([P, 1], F32, tag="rsum")
            nzm = pool.tile([P, 1], F32, tag="nzm")

            zfeat = pool.tile([P, NFEAT], BF16, tag="zfeat")
            zsb = pool.tile([P, NFEAT], BF16, tag="zsb")
            etail = pool.tile([81, 128], BF16, tag="etail")
            nc.sync.dma_start(etail[80:81, :], d_on[:])
            Etok = pool.tile([P, S2], F32, tag="Etok")
            yh0 = pool.tile([128, 128], BF16, tag="yh0")
            yh1 = pool.tile([33, 128], BF16, tag="yh1")
            nc.sync.dma_start(yh1[32:33, :], d_on[:])
            hsb = pool.tile([128, 512], BF16, tag="hsb")
            xext = pool.tile([P, 160], F32, tag="xext")
            dtmp = pool.tile([P, 80], F32, tag="dtmp")
            dsq = pool.tile([P, 80], F32, tag="dsq")

            # initial bf16 shadows + sliding-norm prep for iter 0
            nc.scalar.copy(xpadh[:], xpad[:])
            nc.scalar.copy(yresh[:], yres[:])
            nc.scalar.activation(sqx[:, 1:239], xpad[:], AF.Square)
            nc.vector.tensor_tensor_scan(cs[:], sqx[:],
                                         zero1[:].to_broadcast((P, 239)),
                                         0.0, Op.add, Op.bypass)
            nc.vector.tensor_tensor(nsq[:], cs[:, 80:239], cs[:, 0:159],
                                    Op.subtract)

            def barrel(src, wb, out_w, maxoff):
                """wb[p, j] <- src[p, off_p + j] for j in [0, out_w).

                bits must already hold (off >> b) & 1.  In-place left shifts:
                per-partition masks mean untouched rows keep their values;
                shifted reads are ahead of writes (stream order) so in-place
                is safe.
                """
                nbits = (maxoff).bit_length()
                cw = out_w + maxoff
                nc.vector.tensor_copy(wb[:, 0:cw], src[:, 0:cw])
                for b in range(nbits - 1, -1, -1):
                    sh = 1 << b
                    w = out_w + min(maxoff, 2 * sh - 1) - sh
                    nc.vector.copy_predicated(
                        wb[:, 0:w], bits[:, b:b + 1].to_broadcast((P, w)),
                        wb[:, sh:sh + w])

            def mkbits(src_i32):
                nc.vector.tensor_tensor(
                    bits[:], src_i32[:, 0:1].to_broadcast((P, 8)), i8_t[:],
                    Op.logical_shift_right)
                nc.vector.tensor_tensor(bits[:], bits[:], onesI[:],
                                        Op.bitwise_and)

            for it in range(THINK_ITER):
                # --- dot[p,s] = sum_c xpad[p,s+c]*yres[p,c]: bf16 tree ---
                in0 = bass.AP(xpadh.tensor, xpadh.offset,
                              [list(xpadh.ap[0]), [1, S1], [1, 80]])
                in1 = bass.AP(yresh.tensor, yresh.offset,
                              [list(yresh.ap[0]), [0, S1], [1, 80]])
                dview = dtp[:].rearrange("p (s c) -> p s c", c=80)
                nc.vector.tensor_tensor(dview, in0, in1, Op.mult)
                for (src, dst, w) in ((dtp, t40, 40), (t40, t20, 20),
                                      (t20, t10, 10), (t10, t5, 5)):
                    sv = src[:].rearrange("p (s c) -> p s c", c=2 * w)
                    dv = dst[:].rearrange("p (s c) -> p s c", c=w)
                    nc.vector.tensor_tensor(dv, sv[:, :, 0:w], sv[:, :, w:2 * w],
                                            Op.add)
                nc.vector.tensor_reduce(
                    dot[:], t5[:].rearrange("p (s c) -> p s c", c=5), AX.X,
                    Op.add)
                # --- theta = argmax dot*|dot|/nsq ---
                nc.vector.tensor_scalar_max(rnsq[:], nsq[:], 1e-30)
                nc.vector.reciprocal(rnsq[:], rnsq[:])
                nc.scalar.activation(adot[:], dot[:], AF.Abs)
                nc.vector.tensor_tensor(gsel[:], dot[:], adot[:], Op.mult)
                nc.vector.tensor_tensor(gsel[:], gsel[:], rnsq[:], Op.mult)
                nc.vector.max(mx8[:], gsel[:])
                nc.vector.max_index(mi8[:], mx8[:], gsel[:])
                nc.vector.tensor_copy(thf[:], mi8[:, 0:1])
                # --- y_align barrel gather: yal = wb1[:, 0:80] ---
                nc.vector.tensor_copy(offI[:], mi8[:, 0:1])
                mkbits(offI)
                barrel(xpad, wb1, 80, 158)
                yal = wb1
                # --- softmax attention -> y_att in yap[:, 80:160] ---
                nc.vector.tensor_tensor(zt[:], yal[:, 0:80], yres[:], Op.mult)
                nc.vector.max(mx8[:], zt[:])
                nc.vector.tensor_scalar_mul(nzm[:], mx8[:, 0:1], -1.0 / TEMPER)
                nc.scalar.activation(et[:], zt[:], AF.Exp, bias=nzm[:, 0:1],
                                     scale=1.0 / TEMPER,
                                     accum_out=ssum[:, 0:1])
                nc.vector.reciprocal(rsum[:], ssum[:])
                nc.vector.scalar_tensor_tensor(yap[:, 80:160], et[:],
                                               rsum[:, 0:1], yal[:, 0:80],
                                               Op.mult, Op.mult)
                nc.scalar.copy(yah[:], yap[:, 80:240])
                # --- z features z[p, 80d+i] = ye[i]*ye[i+d] (bf16, one op) ---
                zin0 = bass.AP(yah.tensor, yah.offset,
                               [list(yah.ap[0]), [0, 80], [1, 80]])
                zin1 = bass.AP(yah.tensor, yah.offset,
                               [list(yah.ap[0]), [1, 80], [1, 80]])
                zv = zfeat[:].rearrange("p (d i) -> p d i", i=80)
                nc.vector.tensor_tensor(zv, zin0, zin1, Op.mult)
                # --- tail features: etail = [ya^T (80); ones] ---
                yaTp = pp.tile([128, 128], BF16, tag="tp")
                nc.tensor.transpose(yaTp[0:80, :], yah[:, 0:80], id_t[:])
                nc.scalar.copy(etail[0:80, :], yaTp[0:80, :])
                # --- E accumulation: T (PE) -> copy (S) -> flipped MM (PE) ---
                Eps = ppe.tile([P, S2], mybir.dt.float32, tag="Eps")
                nbatch = (NCHUNK + 3) // 4  # 13 batches of <=4 chunks
                zpss = [None] * nbatch
                for bch in range(nbatch + 1):
                    if bch < nbatch:
                        k0 = bch * 4
                        kn = min(4, NCHUNK - k0)
                        zps = pp.tile([128, 512], BF16, tag="zps")
                        zpss[bch] = (zps, k0, kn)
                        for q in range(kn):
                            k = k0 + q
                            nc.tensor.transpose(
                                zps[:, q * 128:(q + 1) * 128],
                                zfeat[:, k * 128:(k + 1) * 128], id_t[:])
                        nc.scalar.copy(zsb[:, k0 * 128:(k0 + kn) * 128],
                                       zps[:, 0:kn * 128])
                    j = bch - 1
                    if 0 <= j < nbatch:
                        zps, k0, kn = zpss[j]
                        for q in range(kn):
                            k = k0 + q
                            nc.tensor.matmul(
                                Eps[:], zsb[:, k * 128:(k + 1) * 128],
                                A_t[:, k * 81:(k + 1) * 81],
                                start=(k == 0), stop=False)
                nc.tensor.matmul(Eps[:], etail[:], At_t[:], start=False,
                                 stop=True)
                # --- x_ele gather (overlaps E pipe): off = 159 - theta ---
                nc.vector.tensor_scalar(thxf[:], thf[:], -1.0, 159.0,
                                        Op.mult, Op.add)
                nc.vector.tensor_copy(offI[:], thxf[:])
                mkbits(offI)
                barrel(yap, wb2, 80, 159)
                nc.vector.tensor_tensor(xpad[:, 79:159], xpad[:, 79:159],
                                        wb2[:, 0:80], Op.subtract)
                # next-iter prep (overlaps E pipe)
                nc.scalar.copy(xpadh[:], xpad[:])
                nc.scalar.activation(sqx[:, 1:239], xpad[:], AF.Square)
                nc.vector.tensor_tensor_scan(cs[:], sqx[:],
                                             zero1[:].to_broadcast((P, 239)),
                                             0.0, Op.add, Op.bypass)
                nc.vector.tensor_tensor(nsq[:], cs[:, 80:239], cs[:, 0:159],
                                        Op.subtract)
                # --- s* = argmax E (token-major PSUM -> SBUF), d* = 80-s* ---
                nc.scalar.copy(Etok[:], Eps[:])
                nc.vector.max(mx8[:], Etok[:])
                nc.vector.max_index(mi8[:], mx8[:], Etok[:])
                nc.vector.tensor_copy(sff[:], mi8[:, 0:1])
                nc.vector.tensor_scalar(dff[:], sff[:], -1.0, 80.0,
                                        Op.mult, Op.add)
                # --- yhat barrel gather: off = s*, width 160 ---
                nc.vector.tensor_copy(offI[:], mi8[:, 0:1])
                mkbits(offI)
                barrel(yap, wb2, 160, 80)
                nc.scalar.copy(yhath[:], wb2[:, 0:160])
                # --- H = W_enc @ yhat^T + b_enc (h-major, bias folded) ---
                yh0p = pp.tile([128, 128], BF16, tag="tp")
                nc.tensor.transpose(yh0p[:], yhath[:, 0:128], id_t[:])
                nc.scalar.copy(yh0[:], yh0p[:])
                yh1p = pp.tile([128, 128], BF16, tag="tp")
                nc.tensor.transpose(yh1p[0:32, :], yhath[:, 128:160], id_t[:])
                nc.scalar.copy(yh1[0:32, :], yh1p[0:32, :])
                Hps = pph.tile([128, 512], mybir.dt.float32, tag="Hps")
                for hc in range(4):
                    r = slice(hc * 128, (hc + 1) * 128)
                    nc.tensor.matmul(Hps[:, r], We1_t[:, r], yh0[:],
                                     start=True, stop=False)
                    nc.tensor.matmul(Hps[:, r], We2_t[:, r], yh1[:],
                                     start=False, stop=True)
                nc.scalar.copy(hsb[:], Hps[:])
                # --- X = W_src @ h + b_src (token-major via flip) ---
                Xps = ppx.tile([128, 160], mybir.dt.float32, tag="Xps")
                for hc in range(4):
                    nc.tensor.matmul(Xps[:], hsb[:, hc * 128:(hc + 1) * 128],
                                     Ws_t[:, hc * 160:(hc + 1) * 160],
                                     start=(hc == 0), stop=False)
                nc.tensor.matmul(Xps[:], on_t[:], bs_t[:], start=False,
                                 stop=True)
                nc.scalar.copy(xext[:], Xps[:])
                # --- y_ele barrel gather: off = d* = 80 - s* ---
                nc.vector.tensor_copy(offI[:], dff[:])
                mkbits(offI)
                barrel(xext, wb2, 80, 80)
                yele = wb2
                # --- loss partial + state updates ---
                nc.vector.tensor_tensor(dtmp[:], yele[:, 0:80], yres[:],
                                        Op.subtract)
                nc.vector.tensor_tensor(dtmp[:], dtmp[:], keep[:], Op.mult)
                nc.scalar.activation(dsq[:], dtmp[:], AF.Square,
                                     accum_out=lossp[:, it:it + 1])
                nc.vector.tensor_tensor(yres[:], yres[:], yele[:, 0:80],
                                        Op.subtract)
                nc.scalar.copy(yresh[:], yres[:])

            nc.sync.dma_start(d_out[:], lossp[:])
    return nc


def kernel(x, y, W_enc, b_enc, W_src, b_src):
    import sys
    if '/opt/trn_rl_repo' not in sys.path:
        sys.path.insert(0, '/opt/trn_rl_repo')
    x = np.asarray(x, np.float32)
    y = np.asarray(y, np.float32)
    consts = _build_consts(W_enc, b_enc, W_src, b_src)

    if "nc" not in _cache:
        _cache["nc"] = _build_nc()
        _cache["nc"].finalize()
    nc = _cache["nc"]

    xt = x.reshape(NTOK, IDIM)
    yt = y.reshape(NTOK, ODIM)
    in_maps = []
    for c in range(NCORES):
        m = dict(consts)
        m["x"] = np.ascontiguousarray(xt[c * P:(c + 1) * P])
        m["y"] = np.ascontiguousarray(yt[c * P:(c + 1) * P])
        in_maps.append(m)

    from concourse.bass_utils import run_bass_kernel_spmd
    res = run_bass_kernel_spmd(nc, in_maps, list(range(NCORES)))
    parts = np.stack([r["losspart"] for r in res.results])
    keep_cnt = max(int((y != 0.0).sum()), 1)
    nums = parts[:, :, :THINK_ITER].sum(axis=(0, 1), dtype=np.float64)
    losses = (nums / keep_cnt).astype(np.float32)
    return np.float32(np.mean(losses))
